# revision 26
# baseline (speedup 1.0000x reference)
"""Trainium2 Bass kernel for MiniMax softmax attention (T=4096, H=4096, 32 q heads,
8 kv heads, head_dim=128, partial neox RoPE, causal softmax, o_proj).

Sharding: tensor-parallel over heads across 8 NeuronCores. Core c computes q heads
4c..4c+3 (= kv-head group c): qkv^T projection -> RoPE -> causal attention ->
partial o_proj with its w_o row-block. Host sums the 8 partial outputs.

Device layouts (per core):
  hidden_t  [4096 k, 4096 t]  (host-transposed hidden_states)
  w_qkvp    [4096 k, 768 j]   (4 q-head cols * scale, 1 k-head col, 1 v-head col)
  qkv^T     [768 j, 4096 t]   via matmul(psum[j,t], lhsT=w[k,j], rhs=hidden_t[k,t])
  scores^T  [128 key, 512 q]  via matmul(lhsT=kT[d,key], rhs=qT[d,q]); exp on ACT
  softmax denom via ones-column matmul (partition reduce); no max-subtraction
  attn^T    [d, t] via matmul(lhsT=v[key,d], rhs=exp[key,q]); normalized by
  partition-broadcast reciprocal of the denominator
  out_part  [t, o] via matmul(lhsT=attn^T[hd,t], rhs=w_o[hd,o])
All matmuls in float32r (full-rate fp32, ~1e-4 rel err).
"""
import numpy as np

T = 4096
HIDDEN = 4096
NH = 32
NKV = 8
HD = 128
RD = 64
HALF = 32
ROPE_BASE = 10000000.0
NC_CORES = 8
HPC = NH // NC_CORES      # 4 q heads per core
QC = 512                  # query chunk
NTC = T // QC             # 8 t-chunks
NKO = 32                  # hidden contraction chunks of 128
NJ = HPC + 2              # 6 j-tiles of 128 per core

_CACHE = {}


def _build_nc():
    import concourse.mybir as mybir
    import concourse.tile as tile
    from concourse import bacc
    from concourse.masks import make_identity

    F32 = mybir.dt.float32
    F32R = mybir.dt.float32r
    EXP = mybir.ActivationFunctionType.Exp

    nc = bacc.Bacc()
    hidden_t = nc.dram_tensor("hidden_t", [HIDDEN, T], F32R, kind="ExternalInput")
    w_qkvp = nc.dram_tensor("w_qkvp", [HIDDEN, NJ * HD], F32R, kind="ExternalInput")
    w_op = nc.dram_tensor("w_op", [HPC * HD, HIDDEN], F32R, kind="ExternalInput")
    cos_t = nc.dram_tensor("cos_t", [HALF, T], F32, kind="ExternalInput")
    sin_t = nc.dram_tensor("sin_t", [HALF, T], F32, kind="ExternalInput")
    dmask = nc.dram_tensor("dmask", [128, 896], F32, kind="ExternalInput")
    out_p = nc.dram_tensor("out_p", [T, HIDDEN], F32, kind="ExternalOutput")

    with tile.TileContext(nc) as tc:
        with (
            tc.tile_pool(name="const", bufs=1) as const,
            tc.tile_pool(name="kv", bufs=1) as kvp,
            tc.tile_pool(name="spill", bufs=1, space="DRAM") as spillp,
            tc.tile_pool(name="mm", bufs=7, space="PSUM") as mmp,
            tc.tile_pool(name="den", bufs=1, space="PSUM") as denp_pool,
            tc.tile_pool(name="ht", bufs=4) as htp,
            tc.tile_pool(name="qt", bufs=2) as qtp,
            tc.tile_pool(name="rope", bufs=2) as ropep,
            tc.tile_pool(name="vt", bufs=1) as vtp,
            tc.tile_pool(name="ex", bufs=3) as exp_pool,
            tc.tile_pool(name="misc", bufs=2) as miscp,
        ):
            # ---- constants
            cs_sb = const.tile([2 * HALF, T], F32, name="cs", tag="cs")
            cos_sb = cs_sb[:HALF]
            sin_sb = cs_sb[HALF:]
            dmask_sb = const.tile([128, 896], F32, name="dmask", tag="dmask")
            ones_sb = const.tile([128, 1], F32R, name="ones", tag="ones")
            ones_f = const.tile([128, 1], F32, name="ones_f", tag="ones_f")
            ident = const.tile([128, 128], F32, name="ident", tag="ident")
            nc.sync.dma_start(cs_sb[:HALF, :], cos_t[:])
            nc.sync.dma_start(cs_sb[HALF:, :], sin_t[:])
            nc.sync.dma_start(dmask_sb[:], dmask[:])
            nc.gpsimd.memset(ones_f[:], 1.0)
            nc.vector.tensor_copy(ones_sb[:], ones_f[:])
            make_identity(nc, ident)

            kT_tiles = []
            v_tiles = []
            for i in range(NTC):
                kt_i = kvp.tile([128, QC], F32R, name=f"kT{i}", tag=f"kT{i}")
                v_i = kvp.tile([128, 4, 128], F32R, name=f"v{i}", tag=f"v{i}")
                kT_tiles.append(kt_i)
                v_tiles.append(v_i)
            attn_spill = spillp.tile([HPC, 128, T], F32R, name="attn_spill")

            with tc.tile_pool(name="w", bufs=1) as wp:
                w_sb = wp.tile([128, NKO, NJ * HD], F32R, name="w")
                w_view = w_qkvp[:].rearrange("(ko p) j -> p ko j", p=128)
                for wi in range(8):
                    nc.sync.dma_start(
                        w_sb[:, wi * 4:(wi + 1) * 4, :], w_view[:, wi * 4:(wi + 1) * 4, :]
                    )

                for tci in range(NTC):
                    tsl = slice(tci * QC, (tci + 1) * QC)
                    # ---- phase 1: qkv^T for this t-chunk
                    ps_qkv = [
                        mmp.tile([128, QC], F32, name=f"mm{j}", tag="mm") for j in range(NJ)
                    ]
                    for ko2 in range(NKO // 2):
                        ht = htp.tile([128, 2, QC], F32R, name="ht", tag="ht")
                        nc.sync.dma_start(
                            ht[:],
                            hidden_t[ko2 * 256:(ko2 + 1) * 256, tsl].rearrange(
                                "(kk p) t -> p kk t", p=128
                            ),
                        )
                        for kk in range(2):
                            ko = 2 * ko2 + kk
                            for j in range(NJ):
                                nc.tensor.matmul(
                                    ps_qkv[j][:],
                                    w_sb[:, ko, j * HD:(j + 1) * HD],
                                    ht[:, kk, :],
                                    start=(ko == 0),
                                    stop=(ko == NKO - 1),
                                )
                    # ---- evictions: v first (frees a psum slot fastest), then
                    # rope on q0, k (needed by h=0 attention), then q1..q3
                    qcur = qtp.tile([128, HPC, QC], F32R, name="qcur", tag="qt")
                    cos_c = cos_sb[:, tsl]
                    sin_c = sin_sb[:, tsl]
                    vt = vtp.tile([128, QC], F32, name="vt", tag="vt")
                    nc.vector.tensor_copy(vt[:], ps_qkv[HPC + 1][:])
                    for i in range(QC // 128):
                        pt = mmp.tile([128, QC], F32, name="mmt", tag="mm")[:, :128]
                        nc.tensor.transpose(pt[:], vt[:, i * 128:(i + 1) * 128], ident[:])
                        nc.vector.tensor_copy(v_tiles[tci][:, i, :], pt[:])

                    # swapped tables: sin at partitions 0:32, cos at 32:64,
                    # so every rope product has base-partition-aligned inputs
                    css = ropep.tile([RD, QC], F32, name="css", tag="css", bufs=1)
                    nc.sync.dma_start(css[:HALF, :], sin_t[:, tsl])
                    nc.sync.dma_start(css[HALF:, :], cos_t[:, tsl])

                    def _rope_evict(j):
                        # fast full-width ACT copy releases the psum bank,
                        # then in-place rope on SBUF off the critical path
                        ps = ps_qkv[j]
                        dst = qcur[:, j, :] if j < HPC else kT_tiles[tci][:]
                        nc.scalar.copy(dst[:, :], ps[:])
                        x1, x2 = dst[:HALF, :], dst[HALF:RD, :]
                        t1 = ropep.tile([HALF, QC], F32, name="r1", tag="r1", bufs=1)
                        t2 = ropep.tile([HALF, QC], F32, name="r2", tag="r2", bufs=1)
                        t3 = ropep.tile([HALF, QC], F32, name="r3", tag="r3", bufs=1)
                        t4 = ropep.tile([HALF, QC], F32, name="r4", tag="r4", bufs=1)
                        nc.vector.tensor_mul(t1[:], x1, cos_c)
                        nc.vector.tensor_mul(t4[:], x1, css[:HALF, :])
                        nc.vector.tensor_mul(t2[:], x2, sin_c)
                        nc.vector.tensor_sub(x1, t1[:], t2[:])
                        nc.vector.tensor_mul(t3[:], x2, css[HALF:, :])
                        nc.vector.tensor_add(x2, t3[:], t4[:])

                    for j in (0, HPC, 1, 2, 3):
                        _rope_evict(j)

                    # ---- phase 2: attention for q-chunk tci, 4 heads
                    nkt = 4 * tci + 4
                    for h in range(HPC):
                        av = mmp.tile([128, QC], F32, name="av", tag="mm")
                        dn = denp_pool.tile([1, QC], F32, name="dn", tag="dn")
                        for kt in range(nkt):
                            ss = mmp.tile([128, QC], F32, name="ss", tag="mm")
                            nc.tensor.matmul(
                                ss[:],
                                kT_tiles[kt >> 2][:, (kt & 3) * 128:((kt & 3) + 1) * 128],
                                qcur[:, h, :],
                                start=True,
                                stop=True,
                            )
                            ex = exp_pool.tile([128, QC], F32R, name="ex", tag="ex")
                            nc.scalar.activation(ex[:], ss[:], EXP)
                            if kt >= 4 * tci:
                                _o = kt - 4 * tci
                                nc.vector.tensor_mul(
                                    ex[:], ex[:],
                                    dmask_sb[:, 384 - _o * 128:896 - _o * 128],
                                )
                            nc.tensor.matmul(
                                dn[:], ones_sb[:], ex[:],
                                start=(kt == 0), stop=(kt == nkt - 1),
                            )
                            nc.tensor.matmul(
                                av[:], v_tiles[kt >> 2][:, kt & 3, :], ex[:],
                                start=(kt == 0), stop=(kt == nkt - 1),
                            )
                        rd_sb = miscp.tile([1, QC], F32R, name="rd", tag="rd", bufs=1)
                        with nc.allow_low_precision(reason="f32r recip for softmax denom"):
                            nc.vector.reciprocal(rd_sb[:], dn[:])
                        aou = miscp.tile([128, QC], F32, name="aou", tag="aou", bufs=1)
                        nc.scalar.copy(aou[:], av[:])
                        ao = miscp.tile([128, QC], F32R, name="ao", tag="ao")
                        nc.gpsimd.partition_broadcast(ao[:], rd_sb[:])
                        nc.vector.tensor_mul(ao[:], aou[:], ao[:])
                        nc.sync.dma_start(attn_spill[h, :, tsl], ao[:])

            # ---- phase 3: o_proj partial (out_p = attn_part.T @ w_op)
            with tc.tile_pool(name="wo", bufs=1) as wop, \
                 tc.tile_pool(name="p3", bufs=2) as p3p:
                wo_sb = wop.tile([128, HPC, HIDDEN], F32R, name="wo")
                nc.sync.dma_start(
                    wo_sb[:], w_op[:].rearrange("(h d) o -> d h o", d=128)
                )
                for tl in range(T // 128):
                    a_tiles = []
                    for h in range(HPC):
                        at = p3p.tile([128, 128], F32R, name=f"at{h}", tag="at", bufs=16)
                        nc.sync.dma_start(
                            at[:], attn_spill[h, :, tl * 128:(tl + 1) * 128]
                        )
                        a_tiles.append(at)
                    for oc in range(HIDDEN // QC):
                        po = mmp.tile([128, QC], F32, name="po", tag="mm")
                        for h in range(HPC):
                            nc.tensor.matmul(
                                po[:],
                                a_tiles[h][:],
                                wo_sb[:, h, oc * QC:(oc + 1) * QC],
                                start=(h == 0),
                                stop=(h == HPC - 1),
                            )
                        ob = p3p.tile([128, QC], F32, name="ob", tag="ob", bufs=6)
                        nc.vector.tensor_copy(ob[:], po[:])
                        nc.sync.dma_start(
                            out_p[tl * 128:(tl + 1) * 128, oc * QC:(oc + 1) * QC],
                            ob[:],
                        )
    nc.compile()
    return nc


def _host_prep(positions, hidden_states, w_qkv, w_o):
    positions = np.asarray(positions)
    hidden_states = np.asarray(hidden_states, dtype=np.float32)
    w_qkv = np.asarray(w_qkv, dtype=np.float32)
    w_o = np.asarray(w_o, dtype=np.float32)

    hidden_t = np.ascontiguousarray(hidden_states.T)

    pos = positions.astype(np.float32)
    r = np.arange(0, RD, 2, dtype=np.float32) / np.float32(RD)
    inv_freq = (np.float32(1.0) / (np.float32(ROPE_BASE) ** r)).astype(np.float32)
    ang = pos[:, None] * inv_freq[None, :]
    cos_t = np.ascontiguousarray(np.cos(ang).astype(np.float32).T)
    sin_t = np.ascontiguousarray(np.sin(ang).astype(np.float32).T)

    p = np.arange(128, dtype=np.int64)[:, None]
    x = np.arange(896, dtype=np.int64)[None, :]
    dmask = np.ascontiguousarray((x >= p + 384).astype(np.float32))  # [128, 896]

    scale = np.float32(HD ** -0.5)
    q_size = NH * HD
    kv_size = NKV * HD
    in_maps = []
    for c in range(NC_CORES):
        wq = w_qkv[:, c * HPC * HD:(c + 1) * HPC * HD] * scale
        wk = w_qkv[:, q_size + c * HD:q_size + (c + 1) * HD]
        wv = w_qkv[:, q_size + kv_size + c * HD:q_size + kv_size + (c + 1) * HD]
        w_qkvp = np.ascontiguousarray(
            np.concatenate([wq, wk, wv], axis=1), dtype=np.float32
        )
        w_op = np.ascontiguousarray(w_o[c * HPC * HD:(c + 1) * HPC * HD, :])
        in_maps.append(
            {
                "hidden_t": hidden_t,
                "w_qkvp": w_qkvp,
                "w_op": w_op,
                "cos_t": cos_t,
                "sin_t": sin_t,
                "dmask": dmask,
            }
        )
    return in_maps


def kernel(positions, hidden_states, w_qkv, w_o, _trace=False, _trace_kw=None):
    from concourse.bass_utils import run_bass_kernel_spmd

    if "nc" not in _CACHE:
        _CACHE["nc"] = _build_nc()
    nc = _CACHE["nc"]

    in_maps = _host_prep(positions, hidden_states, w_qkv, w_o)
    kw = dict(_trace_kw or {})
    res = run_bass_kernel_spmd(
        nc, in_maps, list(range(NC_CORES)), trace=_trace, **kw
    )
    out = np.zeros((T, HIDDEN), np.float32)
    for c in range(NC_CORES):
        out += res.results[c]["out_p"]
    if _trace:
        _CACHE["last_exec_time_ns"] = res.exec_time_ns
        _CACHE["last_results"] = res
    return out


# revision 27
# speedup vs baseline: 1.0446x; 1.0446x over previous
"""Trainium2 Bass kernel for MiniMax softmax attention (T=4096, H=4096, 32 q heads,
8 kv heads, head_dim=128, partial neox RoPE, causal softmax, o_proj).

Sharding: tensor-parallel over heads across 8 NeuronCores. Core c computes q heads
4c..4c+3 (= kv-head group c): qkv^T projection -> RoPE -> causal attention ->
partial o_proj with its w_o row-block. Host sums the 8 partial outputs.

Device layouts (per core):
  hidden_t  [4096 k, 4096 t]  (host-transposed hidden_states)
  w_qkvp    [4096 k, 768 j]   (4 q-head cols * scale, 1 k-head col, 1 v-head col)
  qkv^T     [768 j, 4096 t]   via matmul(psum[j,t], lhsT=w[k,j], rhs=hidden_t[k,t])
  scores^T  [128 key, 512 q]  via matmul(lhsT=kT[d,key], rhs=qT[d,q]); exp on ACT
  softmax denom via ones-column matmul (partition reduce); no max-subtraction
  attn^T    [d, t] via matmul(lhsT=v[key,d], rhs=exp[key,q]); normalized by
  partition-broadcast reciprocal of the denominator
  out_part  [t, o] via matmul(lhsT=attn^T[hd,t], rhs=w_o[hd,o])
All matmuls in float32r (full-rate fp32, ~1e-4 rel err).
"""
import numpy as np

T = 4096
HIDDEN = 4096
NH = 32
NKV = 8
HD = 128
RD = 64
HALF = 32
ROPE_BASE = 10000000.0
NC_CORES = 8
HPC = NH // NC_CORES      # 4 q heads per core
QC = 512                  # query chunk
NTC = T // QC             # 8 t-chunks
NKO = 32                  # hidden contraction chunks of 128
NJ = HPC + 2              # 6 j-tiles of 128 per core

_CACHE = {}


def _build_nc():
    import concourse.mybir as mybir
    import concourse.tile as tile
    from concourse import bacc
    from concourse.masks import make_identity

    F32 = mybir.dt.float32
    F32R = mybir.dt.float32r
    EXP = mybir.ActivationFunctionType.Exp

    nc = bacc.Bacc()
    hidden_t = nc.dram_tensor("hidden_t", [HIDDEN, T], F32R, kind="ExternalInput")
    w_qkvp = nc.dram_tensor("w_qkvp", [HIDDEN, NJ * HD], F32R, kind="ExternalInput")
    w_op = nc.dram_tensor("w_op", [HPC * HD, HIDDEN], F32R, kind="ExternalInput")
    cos_t = nc.dram_tensor("cos_t", [HALF, T], F32, kind="ExternalInput")
    sin_t = nc.dram_tensor("sin_t", [HALF, T], F32, kind="ExternalInput")
    dmask = nc.dram_tensor("dmask", [128, 896], F32, kind="ExternalInput")
    out_p = nc.dram_tensor("out_p", [T, HIDDEN], F32, kind="ExternalOutput")

    with tile.TileContext(nc) as tc:
        with (
            tc.tile_pool(name="const", bufs=1) as const,
            tc.tile_pool(name="kv", bufs=1) as kvp,
            tc.tile_pool(name="spill", bufs=1, space="DRAM") as spillp,
            tc.tile_pool(name="mm", bufs=6, space="PSUM") as mmp,
            tc.tile_pool(name="den", bufs=2, space="PSUM") as denp_pool,
            tc.tile_pool(name="ht", bufs=4) as htp,
            tc.tile_pool(name="qt", bufs=2) as qtp,
            tc.tile_pool(name="rope", bufs=2) as ropep,
            tc.tile_pool(name="vt", bufs=1) as vtp,
            tc.tile_pool(name="ex", bufs=3) as exp_pool,
            tc.tile_pool(name="misc", bufs=2) as miscp,
        ):
            # ---- constants
            cs_sb = const.tile([2 * HALF, T], F32, name="cs", tag="cs")
            cos_sb = cs_sb[:HALF]
            sin_sb = cs_sb[HALF:]
            dmask_sb = const.tile([128, 896], F32, name="dmask", tag="dmask")
            ones_sb = const.tile([128, 1], F32R, name="ones", tag="ones")
            ones_f = const.tile([128, 1], F32, name="ones_f", tag="ones_f")
            ident = const.tile([128, 128], F32, name="ident", tag="ident")
            nc.sync.dma_start(cs_sb[:HALF, :], cos_t[:])
            nc.sync.dma_start(cs_sb[HALF:, :], sin_t[:])
            nc.sync.dma_start(dmask_sb[:], dmask[:])
            nc.gpsimd.memset(ones_f[:], 1.0)
            nc.vector.tensor_copy(ones_sb[:], ones_f[:])
            make_identity(nc, ident)

            kT_tiles = []
            v_tiles = []
            for i in range(NTC):
                kt_i = kvp.tile([128, QC], F32R, name=f"kT{i}", tag=f"kT{i}")
                v_i = kvp.tile([128, 4, 128], F32R, name=f"v{i}", tag=f"v{i}")
                kT_tiles.append(kt_i)
                v_tiles.append(v_i)
            attn_spill = spillp.tile([HPC, 128, T], F32R, name="attn_spill")

            with tc.tile_pool(name="w", bufs=1) as wp:
                w_sb = wp.tile([128, NKO, NJ * HD], F32R, name="w")
                w_view = w_qkvp[:].rearrange("(ko p) j -> p ko j", p=128)
                for wi in range(8):
                    nc.sync.dma_start(
                        w_sb[:, wi * 4:(wi + 1) * 4, :], w_view[:, wi * 4:(wi + 1) * 4, :]
                    )

                for tci in range(NTC):
                    tsl = slice(tci * QC, (tci + 1) * QC)
                    # ---- phase 1: qkv^T for this t-chunk
                    ps_qkv = [
                        mmp.tile([128, QC], F32, name=f"mm{j}", tag="mm") for j in range(NJ)
                    ]
                    for ko2 in range(NKO // 2):
                        ht = htp.tile([128, 2, QC], F32R, name="ht", tag="ht")
                        nc.sync.dma_start(
                            ht[:],
                            hidden_t[ko2 * 256:(ko2 + 1) * 256, tsl].rearrange(
                                "(kk p) t -> p kk t", p=128
                            ),
                        )
                        for kk in range(2):
                            ko = 2 * ko2 + kk
                            for j in range(NJ):
                                nc.tensor.matmul(
                                    ps_qkv[j][:],
                                    w_sb[:, ko, j * HD:(j + 1) * HD],
                                    ht[:, kk, :],
                                    start=(ko == 0),
                                    stop=(ko == NKO - 1),
                                )
                    # ---- evictions: v first (frees a psum slot fastest), then
                    # rope on q0, k (needed by h=0 attention), then q1..q3
                    qcur = qtp.tile([128, HPC, QC], F32R, name="qcur", tag="qt")
                    cos_c = cos_sb[:, tsl]
                    sin_c = sin_sb[:, tsl]
                    vt = vtp.tile([128, QC], F32, name="vt", tag="vt")
                    nc.vector.tensor_copy(vt[:], ps_qkv[HPC + 1][:])
                    for i in range(QC // 128):
                        pt = mmp.tile([128, QC], F32, name="mmt", tag="mm")[:, :128]
                        nc.tensor.transpose(pt[:], vt[:, i * 128:(i + 1) * 128], ident[:])
                        nc.vector.tensor_copy(v_tiles[tci][:, i, :], pt[:])

                    # swapped tables: sin at partitions 0:32, cos at 32:64,
                    # so every rope product has base-partition-aligned inputs
                    css = ropep.tile([RD, QC], F32, name="css", tag="css", bufs=1)
                    nc.sync.dma_start(css[:HALF, :], sin_t[:, tsl])
                    nc.sync.dma_start(css[HALF:, :], cos_t[:, tsl])

                    def _rope_evict(j):
                        # fast full-width ACT copy releases the psum bank,
                        # then in-place rope on SBUF off the critical path
                        ps = ps_qkv[j]
                        dst = qcur[:, j, :] if j < HPC else kT_tiles[tci][:]
                        nc.scalar.copy(dst[:, :], ps[:])
                        x1, x2 = dst[:HALF, :], dst[HALF:RD, :]
                        t1 = ropep.tile([HALF, QC], F32, name="r1", tag="r1", bufs=1)
                        t2 = ropep.tile([HALF, QC], F32, name="r2", tag="r2", bufs=1)
                        t3 = ropep.tile([HALF, QC], F32, name="r3", tag="r3", bufs=1)
                        t4 = ropep.tile([HALF, QC], F32, name="r4", tag="r4", bufs=1)
                        nc.vector.tensor_mul(t1[:], x1, cos_c)
                        nc.vector.tensor_mul(t4[:], x1, css[:HALF, :])
                        nc.vector.tensor_mul(t2[:], x2, sin_c)
                        nc.vector.tensor_sub(x1, t1[:], t2[:])
                        nc.vector.tensor_mul(t3[:], x2, css[HALF:, :])
                        nc.vector.tensor_add(x2, t3[:], t4[:])

                    for j in (0, HPC, 1, 2, 3):
                        _rope_evict(j)

                    # ---- phase 2: attention for q-chunk tci, 4 heads
                    nkt = 4 * tci + 4
                    for h in range(HPC):
                        av = mmp.tile([128, QC], F32, name="av", tag="mm")
                        dn = denp_pool.tile([1, QC], F32, name="dn", tag="dn")
                        for kt in range(nkt):
                            ss = mmp.tile([128, QC], F32, name="ss", tag="mm")
                            nc.tensor.matmul(
                                ss[:],
                                kT_tiles[kt >> 2][:, (kt & 3) * 128:((kt & 3) + 1) * 128],
                                qcur[:, h, :],
                                start=True,
                                stop=True,
                            )
                            ex = exp_pool.tile([128, QC], F32R, name="ex", tag="ex")
                            nc.scalar.activation(ex[:], ss[:], EXP)
                            if kt >= 4 * tci:
                                _o = kt - 4 * tci
                                nc.vector.tensor_mul(
                                    ex[:], ex[:],
                                    dmask_sb[:, 384 - _o * 128:896 - _o * 128],
                                )
                            nc.tensor.matmul(
                                dn[:], ones_sb[:], ex[:],
                                start=(kt == 0), stop=(kt == nkt - 1),
                            )
                            nc.tensor.matmul(
                                av[:], v_tiles[kt >> 2][:, kt & 3, :], ex[:],
                                start=(kt == 0), stop=(kt == nkt - 1),
                            )
                        rd_sb = miscp.tile([1, QC], F32R, name="rd", tag="rd", bufs=1)
                        with nc.allow_low_precision(reason="f32r recip for softmax denom"):
                            nc.vector.reciprocal(rd_sb[:], dn[:])
                        aou = miscp.tile([128, QC], F32, name="aou", tag="aou", bufs=1)
                        nc.scalar.copy(aou[:], av[:])
                        ao = miscp.tile([128, QC], F32R, name="ao", tag="ao")
                        nc.gpsimd.partition_broadcast(ao[:], rd_sb[:])
                        nc.vector.tensor_mul(ao[:], aou[:], ao[:])
                        nc.sync.dma_start(attn_spill[h, :, tsl], ao[:])

            # ---- phase 3: o_proj partial (out_p = attn_part.T @ w_op)
            with tc.tile_pool(name="wo", bufs=1) as wop, \
                 tc.tile_pool(name="p3", bufs=2) as p3p:
                wo_sb = wop.tile([128, HPC, HIDDEN], F32R, name="wo")
                nc.sync.dma_start(
                    wo_sb[:], w_op[:].rearrange("(h d) o -> d h o", d=128)
                )
                for tl in range(T // 128):
                    a_tiles = []
                    for h in range(HPC):
                        at = p3p.tile([128, 128], F32R, name=f"at{h}", tag="at", bufs=16)
                        nc.sync.dma_start(
                            at[:], attn_spill[h, :, tl * 128:(tl + 1) * 128]
                        )
                        a_tiles.append(at)
                    for oc in range(HIDDEN // QC):
                        po = mmp.tile([128, QC], F32, name="po", tag="mm")
                        for h in range(HPC):
                            nc.tensor.matmul(
                                po[:],
                                a_tiles[h][:],
                                wo_sb[:, h, oc * QC:(oc + 1) * QC],
                                start=(h == 0),
                                stop=(h == HPC - 1),
                            )
                        ob = p3p.tile([128, QC], F32, name="ob", tag="ob", bufs=6)
                        nc.vector.tensor_copy(ob[:], po[:])
                        nc.sync.dma_start(
                            out_p[tl * 128:(tl + 1) * 128, oc * QC:(oc + 1) * QC],
                            ob[:],
                        )
    nc.compile()
    return nc


def _host_prep(positions, hidden_states, w_qkv, w_o):
    positions = np.asarray(positions)
    hidden_states = np.asarray(hidden_states, dtype=np.float32)
    w_qkv = np.asarray(w_qkv, dtype=np.float32)
    w_o = np.asarray(w_o, dtype=np.float32)

    hidden_t = np.ascontiguousarray(hidden_states.T)

    pos = positions.astype(np.float32)
    r = np.arange(0, RD, 2, dtype=np.float32) / np.float32(RD)
    inv_freq = (np.float32(1.0) / (np.float32(ROPE_BASE) ** r)).astype(np.float32)
    ang = pos[:, None] * inv_freq[None, :]
    cos_t = np.ascontiguousarray(np.cos(ang).astype(np.float32).T)
    sin_t = np.ascontiguousarray(np.sin(ang).astype(np.float32).T)

    p = np.arange(128, dtype=np.int64)[:, None]
    x = np.arange(896, dtype=np.int64)[None, :]
    dmask = np.ascontiguousarray((x >= p + 384).astype(np.float32))  # [128, 896]

    scale = np.float32(HD ** -0.5)
    q_size = NH * HD
    kv_size = NKV * HD
    in_maps = []
    for c in range(NC_CORES):
        wq = w_qkv[:, c * HPC * HD:(c + 1) * HPC * HD] * scale
        wk = w_qkv[:, q_size + c * HD:q_size + (c + 1) * HD]
        wv = w_qkv[:, q_size + kv_size + c * HD:q_size + kv_size + (c + 1) * HD]
        w_qkvp = np.ascontiguousarray(
            np.concatenate([wq, wk, wv], axis=1), dtype=np.float32
        )
        w_op = np.ascontiguousarray(w_o[c * HPC * HD:(c + 1) * HPC * HD, :])
        in_maps.append(
            {
                "hidden_t": hidden_t,
                "w_qkvp": w_qkvp,
                "w_op": w_op,
                "cos_t": cos_t,
                "sin_t": sin_t,
                "dmask": dmask,
            }
        )
    return in_maps


def kernel(positions, hidden_states, w_qkv, w_o, _trace=False, _trace_kw=None):
    from concourse.bass_utils import run_bass_kernel_spmd

    if "nc" not in _CACHE:
        _CACHE["nc"] = _build_nc()
    nc = _CACHE["nc"]

    in_maps = _host_prep(positions, hidden_states, w_qkv, w_o)
    kw = dict(_trace_kw or {})
    res = run_bass_kernel_spmd(
        nc, in_maps, list(range(NC_CORES)), trace=_trace, **kw
    )
    out = np.zeros((T, HIDDEN), np.float32)
    for c in range(NC_CORES):
        out += res.results[c]["out_p"]
    if _trace:
        _CACHE["last_exec_time_ns"] = res.exec_time_ns
        _CACHE["last_results"] = res
    return out


# revision 28
# speedup vs baseline: 1.0672x; 1.0217x over previous
"""Trainium2 Bass kernel for MiniMax softmax attention (T=4096, H=4096, 32 q heads,
8 kv heads, head_dim=128, partial neox RoPE, causal softmax, o_proj).

Sharding: tensor-parallel over heads across 8 NeuronCores. Core c computes q heads
4c..4c+3 (= kv-head group c): qkv^T projection -> RoPE -> causal attention ->
partial o_proj with its w_o row-block. Host sums the 8 partial outputs.

Device layouts (per core):
  hidden_t  [4096 k, 4096 t]  (host-transposed hidden_states)
  w_qkvp    [4096 k, 768 j]   (4 q-head cols * scale, 1 k-head col, 1 v-head col)
  qkv^T     [768 j, 4096 t]   via matmul(psum[j,t], lhsT=w[k,j], rhs=hidden_t[k,t])
  scores^T  [128 key, 512 q]  via matmul(lhsT=kT[d,key], rhs=qT[d,q]); exp on ACT
  softmax denom via ones-column matmul (partition reduce); no max-subtraction
  attn^T    [d, t] via matmul(lhsT=v[key,d], rhs=exp[key,q]); normalized by
  partition-broadcast reciprocal of the denominator
  out_part  [t, o] via matmul(lhsT=attn^T[hd,t], rhs=w_o[hd,o])
All matmuls in float32r (full-rate fp32, ~1e-4 rel err).
"""
import numpy as np

T = 4096
HIDDEN = 4096
NH = 32
NKV = 8
HD = 128
RD = 64
HALF = 32
ROPE_BASE = 10000000.0
NC_CORES = 8
HPC = NH // NC_CORES      # 4 q heads per core
QC = 512                  # query chunk
NTC = T // QC             # 8 t-chunks
NKO = 32                  # hidden contraction chunks of 128
NJ = HPC + 2              # 6 j-tiles of 128 per core

_CACHE = {}


def _build_nc():
    import concourse.mybir as mybir
    import concourse.tile as tile
    from concourse import bacc
    from concourse.masks import make_identity

    F32 = mybir.dt.float32
    F32R = mybir.dt.float32r
    EXP = mybir.ActivationFunctionType.Exp

    nc = bacc.Bacc()
    hidden_t = nc.dram_tensor("hidden_t", [HIDDEN, T], F32R, kind="ExternalInput")
    w_qkvp = nc.dram_tensor("w_qkvp", [HIDDEN, NJ * HD], F32R, kind="ExternalInput")
    w_op = nc.dram_tensor("w_op", [HPC * HD, HIDDEN], F32R, kind="ExternalInput")
    cos_t = nc.dram_tensor("cos_t", [HALF, T], F32, kind="ExternalInput")
    sin_t = nc.dram_tensor("sin_t", [HALF, T], F32, kind="ExternalInput")
    dmask = nc.dram_tensor("dmask", [128, 896], F32, kind="ExternalInput")
    out_p = nc.dram_tensor("out_p", [T, HIDDEN], F32, kind="ExternalOutput")

    with tile.TileContext(nc) as tc:
        with (
            tc.tile_pool(name="const", bufs=1) as const,
            tc.tile_pool(name="kv", bufs=1) as kvp,
            tc.tile_pool(name="spill", bufs=1, space="DRAM") as spillp,
            tc.tile_pool(name="mm", bufs=6, space="PSUM") as mmp,
            tc.tile_pool(name="den", bufs=2, space="PSUM") as denp_pool,
            tc.tile_pool(name="ht", bufs=4) as htp,
            tc.tile_pool(name="qt", bufs=2) as qtp,
            tc.tile_pool(name="rope", bufs=2) as ropep,
            tc.tile_pool(name="vt", bufs=1) as vtp,
            tc.tile_pool(name="ex", bufs=3) as exp_pool,
            tc.tile_pool(name="misc", bufs=2) as miscp,
        ):
            # ---- constants
            cs_sb = const.tile([2 * HALF, T], F32, name="cs", tag="cs")
            cos_sb = cs_sb[:HALF]
            sin_sb = cs_sb[HALF:]
            dmask_sb = const.tile([128, 896], F32, name="dmask", tag="dmask")
            ones_sb = const.tile([128, 1], F32R, name="ones", tag="ones")
            ones_f = const.tile([128, 1], F32, name="ones_f", tag="ones_f")
            ident = const.tile([128, 128], F32, name="ident", tag="ident")
            nc.sync.dma_start(cs_sb[:HALF, :], cos_t[:])
            nc.sync.dma_start(cs_sb[HALF:, :], sin_t[:])
            nc.sync.dma_start(dmask_sb[:], dmask[:])
            nc.gpsimd.memset(ones_f[:], 1.0)
            nc.vector.tensor_copy(ones_sb[:], ones_f[:])
            make_identity(nc, ident)

            kT_tiles = []
            v_tiles = []
            for i in range(NTC):
                kt_i = kvp.tile([128, QC], F32R, name=f"kT{i}", tag=f"kT{i}")
                v_i = kvp.tile([128, 4, 128], F32R, name=f"v{i}", tag=f"v{i}")
                kT_tiles.append(kt_i)
                v_tiles.append(v_i)
            attn_spill = spillp.tile([HPC, 128, T], F32R, name="attn_spill")

            with tc.tile_pool(name="w", bufs=1) as wp:
                w_sb = wp.tile([128, NKO, NJ * HD], F32R, name="w")
                w_view = w_qkvp[:].rearrange("(ko p) j -> p ko j", p=128)
                for wi in range(8):
                    nc.sync.dma_start(
                        w_sb[:, wi * 4:(wi + 1) * 4, :], w_view[:, wi * 4:(wi + 1) * 4, :]
                    )

                for tci in range(NTC):
                    tsl = slice(tci * QC, (tci + 1) * QC)
                    # ---- phase 1: qkv^T for this t-chunk
                    ps_qkv = [
                        mmp.tile([128, QC], F32, name=f"mm{j}", tag="mm") for j in range(NJ)
                    ]
                    for ko2 in range(NKO // 2):
                        ht = htp.tile([128, 2, QC], F32R, name="ht", tag="ht")
                        nc.sync.dma_start(
                            ht[:],
                            hidden_t[ko2 * 256:(ko2 + 1) * 256, tsl].rearrange(
                                "(kk p) t -> p kk t", p=128
                            ),
                        )
                        for kk in range(2):
                            ko = 2 * ko2 + kk
                            for j in range(NJ):
                                nc.tensor.matmul(
                                    ps_qkv[j][:],
                                    w_sb[:, ko, j * HD:(j + 1) * HD],
                                    ht[:, kk, :],
                                    start=(ko == 0),
                                    stop=(ko == NKO - 1),
                                )
                    # ---- evictions: v first (frees a psum slot fastest), then
                    # rope on q0, k (needed by h=0 attention), then q1..q3
                    qcur = qtp.tile([128, HPC, QC], F32R, name="qcur", tag="qt")
                    cos_c = cos_sb[:, tsl]
                    sin_c = sin_sb[:, tsl]
                    vt = vtp.tile([128, QC], F32, name="vt", tag="vt")
                    nc.vector.tensor_copy(vt[:], ps_qkv[HPC + 1][:])
                    for i in range(QC // 128):
                        pt = mmp.tile([128, QC], F32, name="mmt", tag="mm")[:, :128]
                        nc.tensor.transpose(pt[:], vt[:, i * 128:(i + 1) * 128], ident[:])
                        nc.vector.tensor_copy(v_tiles[tci][:, i, :], pt[:])

                    # swapped tables: sin at partitions 0:32, cos at 32:64,
                    # so every rope product has base-partition-aligned inputs
                    css = ropep.tile([RD, QC], F32, name="css", tag="css", bufs=1)
                    nc.sync.dma_start(css[:HALF, :], sin_t[:, tsl])
                    nc.sync.dma_start(css[HALF:, :], cos_t[:, tsl])

                    def _rope_evict(j):
                        # fast full-width ACT copy releases the psum bank,
                        # then in-place rope on SBUF off the critical path
                        ps = ps_qkv[j]
                        dst = qcur[:, j, :] if j < HPC else kT_tiles[tci][:]
                        nc.scalar.copy(dst[:, :], ps[:])
                        x1, x2 = dst[:HALF, :], dst[HALF:RD, :]
                        t1 = ropep.tile([HALF, QC], F32, name="r1", tag="r1", bufs=1)
                        t2 = ropep.tile([HALF, QC], F32, name="r2", tag="r2", bufs=1)
                        t3 = ropep.tile([HALF, QC], F32, name="r3", tag="r3", bufs=1)
                        t4 = ropep.tile([HALF, QC], F32, name="r4", tag="r4", bufs=1)
                        nc.vector.tensor_mul(t1[:], x1, cos_c)
                        nc.vector.tensor_mul(t4[:], x1, css[:HALF, :])
                        nc.vector.tensor_mul(t2[:], x2, sin_c)
                        nc.vector.tensor_sub(x1, t1[:], t2[:])
                        nc.vector.tensor_mul(t3[:], x2, css[HALF:, :])
                        nc.vector.tensor_add(x2, t3[:], t4[:])

                    for j in (0, HPC, 1, 2, 3):
                        _rope_evict(j)

                    # ---- phase 2: attention for q-chunk tci, 4 heads
                    nkt = 4 * tci + 4
                    for h in range(HPC):
                        av = mmp.tile([128, QC], F32, name="av", tag="mm")
                        dn = denp_pool.tile([1, QC], F32, name="dn", tag="dn")
                        for kt in range(nkt):
                            # diagonal tiles: only queries >= o*128 are unmasked;
                            # restrict the moving range (min 256 to keep f32r full rate)
                            _o = kt - 4 * tci
                            qoff = 0 if _o < 0 else min(_o * 128, QC - 256)
                            qs = slice(qoff, QC)
                            ss = mmp.tile([128, QC], F32, name="ss", tag="mm")
                            nc.tensor.matmul(
                                ss[:, qs],
                                kT_tiles[kt >> 2][:, (kt & 3) * 128:((kt & 3) + 1) * 128],
                                qcur[:, h, qs],
                                start=True,
                                stop=True,
                            )
                            ex = exp_pool.tile([128, QC], F32R, name="ex", tag="ex")
                            nc.scalar.activation(ex[:, qs], ss[:, qs], EXP)
                            if _o >= 0:
                                _off = _o * 128 - qoff
                                nc.vector.tensor_mul(
                                    ex[:, qs], ex[:, qs],
                                    dmask_sb[:, 384 - _off:384 - _off + (QC - qoff)],
                                )
                            nc.tensor.matmul(
                                dn[:, qs], ones_sb[:], ex[:, qs],
                                start=(kt == 0), stop=(kt == nkt - 1),
                            )
                            nc.tensor.matmul(
                                av[:, qs], v_tiles[kt >> 2][:, kt & 3, :], ex[:, qs],
                                start=(kt == 0), stop=(kt == nkt - 1),
                            )
                        rd_sb = miscp.tile([1, QC], F32R, name="rd", tag="rd", bufs=1)
                        with nc.allow_low_precision(reason="f32r recip for softmax denom"):
                            nc.vector.reciprocal(rd_sb[:], dn[:])
                        aou = miscp.tile([128, QC], F32, name="aou", tag="aou", bufs=1)
                        nc.scalar.copy(aou[:], av[:])
                        ao = miscp.tile([128, QC], F32R, name="ao", tag="ao")
                        nc.gpsimd.partition_broadcast(ao[:], rd_sb[:])
                        nc.vector.tensor_mul(ao[:], aou[:], ao[:])
                        nc.sync.dma_start(attn_spill[h, :, tsl], ao[:])

            # ---- phase 3: o_proj partial (out_p = attn_part.T @ w_op)
            with tc.tile_pool(name="wo", bufs=1) as wop, \
                 tc.tile_pool(name="p3", bufs=2) as p3p:
                wo_sb = wop.tile([128, HPC, HIDDEN], F32R, name="wo")
                nc.sync.dma_start(
                    wo_sb[:], w_op[:].rearrange("(h d) o -> d h o", d=128)
                )
                for tl in range(T // 128):
                    a_tiles = []
                    for h in range(HPC):
                        at = p3p.tile([128, 128], F32R, name=f"at{h}", tag="at", bufs=16)
                        nc.sync.dma_start(
                            at[:], attn_spill[h, :, tl * 128:(tl + 1) * 128]
                        )
                        a_tiles.append(at)
                    for oc in range(HIDDEN // QC):
                        po = mmp.tile([128, QC], F32, name="po", tag="mm")
                        for h in range(HPC):
                            nc.tensor.matmul(
                                po[:],
                                a_tiles[h][:],
                                wo_sb[:, h, oc * QC:(oc + 1) * QC],
                                start=(h == 0),
                                stop=(h == HPC - 1),
                            )
                        ob = p3p.tile([128, QC], F32, name="ob", tag="ob", bufs=6)
                        nc.vector.tensor_copy(ob[:], po[:])
                        nc.sync.dma_start(
                            out_p[tl * 128:(tl + 1) * 128, oc * QC:(oc + 1) * QC],
                            ob[:],
                        )
    nc.compile()
    return nc


def _host_prep(positions, hidden_states, w_qkv, w_o):
    positions = np.asarray(positions)
    hidden_states = np.asarray(hidden_states, dtype=np.float32)
    w_qkv = np.asarray(w_qkv, dtype=np.float32)
    w_o = np.asarray(w_o, dtype=np.float32)

    hidden_t = np.ascontiguousarray(hidden_states.T)

    pos = positions.astype(np.float32)
    r = np.arange(0, RD, 2, dtype=np.float32) / np.float32(RD)
    inv_freq = (np.float32(1.0) / (np.float32(ROPE_BASE) ** r)).astype(np.float32)
    ang = pos[:, None] * inv_freq[None, :]
    cos_t = np.ascontiguousarray(np.cos(ang).astype(np.float32).T)
    sin_t = np.ascontiguousarray(np.sin(ang).astype(np.float32).T)

    p = np.arange(128, dtype=np.int64)[:, None]
    x = np.arange(896, dtype=np.int64)[None, :]
    dmask = np.ascontiguousarray((x >= p + 384).astype(np.float32))  # [128, 896]

    scale = np.float32(HD ** -0.5)
    q_size = NH * HD
    kv_size = NKV * HD
    in_maps = []
    for c in range(NC_CORES):
        wq = w_qkv[:, c * HPC * HD:(c + 1) * HPC * HD] * scale
        wk = w_qkv[:, q_size + c * HD:q_size + (c + 1) * HD]
        wv = w_qkv[:, q_size + kv_size + c * HD:q_size + kv_size + (c + 1) * HD]
        w_qkvp = np.ascontiguousarray(
            np.concatenate([wq, wk, wv], axis=1), dtype=np.float32
        )
        w_op = np.ascontiguousarray(w_o[c * HPC * HD:(c + 1) * HPC * HD, :])
        in_maps.append(
            {
                "hidden_t": hidden_t,
                "w_qkvp": w_qkvp,
                "w_op": w_op,
                "cos_t": cos_t,
                "sin_t": sin_t,
                "dmask": dmask,
            }
        )
    return in_maps


def kernel(positions, hidden_states, w_qkv, w_o, _trace=False, _trace_kw=None):
    from concourse.bass_utils import run_bass_kernel_spmd

    if "nc" not in _CACHE:
        _CACHE["nc"] = _build_nc()
    nc = _CACHE["nc"]

    in_maps = _host_prep(positions, hidden_states, w_qkv, w_o)
    kw = dict(_trace_kw or {})
    res = run_bass_kernel_spmd(
        nc, in_maps, list(range(NC_CORES)), trace=_trace, **kw
    )
    out = np.zeros((T, HIDDEN), np.float32)
    for c in range(NC_CORES):
        out += res.results[c]["out_p"]
    if _trace:
        _CACHE["last_exec_time_ns"] = res.exec_time_ns
        _CACHE["last_results"] = res
    return out
